# revision 1
# baseline (speedup 1.0000x reference)
"""Trainium2 Bass kernel for a GPT-style causal attention block.

  y = proj( softmax_causal( (x@Wq)(x@Wk)^T / sqrt(hd) ) @ (x@Wv) )

Shapes (hardcoded): B=2, S=2048, D=1024, H=16 heads, hd=64.

Sharding over 8 NeuronCores: core = (batch b, head-group g), g selects 4
heads. Each core:
  phase 1: QKV projection for its 4 heads (fp32r matmuls).
           q,k produced TRANSPOSED  [head_ch, S]  (contraction-ready),
           v produced natural       [S, head_ch] (+ a ones column).
  phase 2: causal attention, head PAIRS processed together in the
           transposed-score layout [key, query]: the two K=64 score
           matmuls of a pair run CONCURRENTLY in the PE array via
           row-group tile_position (0,0)/(64,0); exp on ACT (scale=1/8
           folded in); causal mask multiplies only the 4 diagonal key
           tiles (emitted first so they overlap the non-diagonal
           stream; split between DVE and gpsimd); AV matmul with
           lhsT=[v|1] so psum row 64 accumulates the softmax
           denominator; normalization via a PE-replicated reciprocal.
  phase 3: AllGather of aT (bf16, transposed) across the 4 cores of the
           same batch, then output projection column-shard: core
           computes y[:, g*256:(g+1)*256] for its batch (bf16 weights).

Matmuls run in float32r (full PE rate when the moving free dim is
>=256; fp32 would be 4x slower). All host-side sharding/layout prep is
data-only so the single SPMD program is rank-independent (the
collective firmware handles rank placement inside replica_groups).
Measured on HW: relative L2 error ~2.4e-3 vs the fp32 reference
(float32r matmuls ~2.8e-4 + bf16 AllGather payload).
"""

import numpy as np

B = 2
S = 2048
D = 1024
H = 16
HD = 64
HLOC = 4          # heads per core
NPAIR = 2         # head pairs per core
N_CORES = 8
GROUP = 4         # cores per batch (replica group size)
IB = 512          # query block width (matmul moving dim)
JT = 128          # key tile (psum partition dim)
OC = D // GROUP   # output-projection column shard per core (256)
SCALE = 1.0 / 8.0  # 1/sqrt(hd)


def _build_bass(s=S, repeat=1, phases="all"):
    """Build the SPMD Bass program (one NeuronCore's view). `s` is the
    sequence length (parameterized only so a small config can run in the
    simulator). `repeat` re-runs the whole computation N times inside one
    NEFF (used to measure device time net of dispatch overhead)."""
    import concourse.bacc as bacc
    import concourse.mybir as mybir
    import concourse.tile as tile

    f32 = mybir.dt.float32
    f32r = mybir.dt.float32r
    bf16 = mybir.dt.bfloat16
    Alu = mybir.AluOpType
    Act = mybir.ActivationFunctionType

    n_ib = s // IB           # query blocks
    n_st = s // 128          # 128-row sequence tiles
    n_dt = D // 128          # contraction tiles for D

    # Bacc (not plain Bass): its compile() lowers multi-wait sync_infos into
    # event-semaphore nops, which walrus codegen requires.
    nc = bacc.Bacc(num_devices=N_CORES)

    xt = nc.declare_dram_parameter("xt", [D, s], f32r, isOutput=False)
    wqk = nc.declare_dram_parameter("wqk", [D, 512], f32r, isOutput=False)
    wv = nc.declare_dram_parameter("wv", [D, 256], f32r, isOutput=False)
    bqk = nc.declare_dram_parameter("bqk", [128, 4], f32, isOutput=False)
    bv = nc.declare_dram_parameter("bv", [128, 256], f32, isOutput=False)
    wp = nc.declare_dram_parameter("wp", [D, OC], bf16, isOutput=False)
    bp = nc.declare_dram_parameter("bp", [128, OC], f32, isOutput=False)
    msk = nc.declare_dram_parameter("msk", [128, 4, IB], f32, isOutput=False)
    # all-ones constants (f32r memset fails the walrus ISA check, so DMA them)
    one_r = nc.declare_dram_parameter("one_r", [128, 65], f32r, isOutput=False)
    # selector for the pairwise reciprocal replicate matmul:
    # sel2[0,0:64]=1, sel2[1,64:128]=1
    sel2 = nc.declare_dram_parameter("sel2", [2, 128], f32r, isOutput=False)
    y = nc.declare_dram_parameter("y", [s, OC], f32, isOutput=True)

    with tile.TileContext(nc) as tc:
        with (
            tc.tile_pool(name="const", bufs=1) as const,
            tc.tile_pool(name="persist", bufs=1) as persist,
            tc.tile_pool(name="dram", bufs=1, space="DRAM") as dram,
        ):
            bqk_sb = const.tile([128, 4], f32)
            nc.sync.dma_start(out=bqk_sb, in_=bqk[:, :])
            bv_sb = const.tile([128, 256], f32)
            nc.sync.dma_start(out=bv_sb, in_=bv[:, :])
            bp_sb = const.tile([128, OC], f32)
            nc.sync.dma_start(out=bp_sb, in_=bp[:, :])
            msk_sb = const.tile([128, 4, IB], f32)
            nc.sync.dma_start(out=msk_sb, in_=msk[:, :, :])
            wp_sb = const.tile([128, n_dt, OC], bf16)
            nc.sync.dma_start(
                out=wp_sb, in_=wp.rearrange("(t p) c -> p t c", p=128)
            )
            sel2_sb = const.tile([2, 128], f32r)
            nc.sync.dma_start(out=sel2_sb, in_=sel2[:, :])
            # dummy exp: pulls the ACT exp table load off the critical path
            warm_sb = const.tile([1, 1], f32)
            nc.scalar.activation(
                out=warm_sb, in_=bqk_sb[0:1, 0:1], func=Act.Exp, scale=0.0
            )

            # persistent intermediates
            for _rep in range(repeat):
             qT_sb = persist.tile([128, NPAIR, s], f32r, name="qT_sb")   # [pair_ch, pair, s]
             kT_sb = persist.tile([128, NPAIR, s], f32r)
             v_sb = persist.tile([128, n_st, HLOC, 65], f32r)  # [:, st, h, 64]=ones
             aT_sb = persist.tile([128, NPAIR, s], bf16)

             ag_in = dram.tile([256, s], bf16, name="ag_in")
             # NB: addr_space="Shared" is rejected for 4-core replica groups.
             ag_out = dram.tile([1024, s], bf16, name="ag_out")

             for st in range(n_st):
                 nc.sync.dma_start(
                     out=v_sb[:, st, :, 64:65],
                     in_=one_r[:, 0:HLOC].rearrange("p (h o) -> p h o", o=1),
                 )

             # ------- phase 1 + 2: QKV projection interleaved with attention.
             # V first, then per head-pair: that pair's q/k projection
             # followed immediately by its attention — so the ACT exp
             # stream (the attention-phase tail) starts ~14us earlier
             # than a fully phased schedule.
             with (
                 tc.tile_pool(name="p1in", bufs=1) as p1in,
             ):
                 xt_sb = p1in.tile([128, n_dt, s], f32r)
                 for sh in range(2):
                     for t in range(n_dt):
                         nc.sync.dma_start(
                             out=xt_sb[:, t, sh * s // 2 : (sh + 1) * s // 2],
                             in_=xt.rearrange("(t p) ss -> p t ss", p=128)[
                                 :, t, sh * s // 2 : (sh + 1) * s // 2
                             ],
                         )
                 wqk_sb = p1in.tile([128, n_dt, 512], f32r)
                 nc.sync.dma_start(
                     out=wqk_sb, in_=wqk.rearrange("(t p) c -> p t c", p=128)
                 )
                 wv_sb = p1in.tile([128, n_dt, 256], f32r)
                 nc.sync.dma_start(
                     out=wv_sb, in_=wv.rearrange("(t p) c -> p t c", p=128)
                 )

                 # v natural: lhsT = xT tile [d, s-tile], rhs = Wv [d, 256]
                 def v_for(st_lo, st_hi, pool):
                     for st in range(st_lo, st_hi):
                         psv = pool.tile([128, 256], f32, name="psv", tag="pss")
                         for dt in range(n_dt):
                             nc.tensor.matmul(
                                 psv,
                                 lhsT=(xt_sb[:, dt, st * 128 : (st + 1) * 128]),
                                 rhs=(wv_sb[:, dt, :]),
                                 start=(dt == 0),
                                 stop=(dt == n_dt - 1),
                             )
                         nc.vector.tensor_tensor(
                             out=v_sb[:, st, :, 0:64],
                             in0=psv.rearrange("p (h e) -> p h e", h=HLOC),
                             in1=bv_sb.rearrange("p (h e) -> p h e", h=HLOC),
                             op=Alu.add,
                         )

                 def qkT_for(t, pool):
                     # qT/kT: lhsT = W tile [d,c], rhs = xT [d, s-block]
                     # c-tile t: 0,1 = q pair0/1; 2,3 = k pair0/1
                     # (psum comes from the shared scores pool slots)
                     for sb in range(n_ib):
                         ps = pool.tile([128, IB], f32, name="ps", tag="pss")
                         for dt in range(n_dt):
                             nc.tensor.matmul(
                                 ps,
                                 lhsT=(wqk_sb[:, dt, t * 128 : (t + 1) * 128]),
                                 rhs=(xt_sb[:, dt, sb * IB : (sb + 1) * IB]),
                                 start=(dt == 0),
                                 stop=(dt == n_dt - 1),
                             )
                         dst = qT_sb if t < 2 else kT_sb
                         nc.vector.tensor_scalar_add(
                             out=dst[:, t % 2, sb * IB : (sb + 1) * IB],
                             in0=ps,
                             scalar1=bqk_sb[:, t : t + 1],
                         )

                 if phases == "p1":
                     with tc.tile_pool(
                         name="ps_p1", bufs=2, space="PSUM"
                     ) as ps_p1:
                         v_for(0, n_st, ps_p1)
                         for t in range(4):
                             qkT_for(t, ps_p1)
                     continue
                 # ---- attention: head PAIRS, scores row-group packed ----
                 with (
                     tc.tile_pool(name="ps_s", bufs=2, space="PSUM") as ps_s,
                     tc.tile_pool(name="ps_av", bufs=2, space="PSUM") as ps_av,
                     tc.tile_pool(name="pt", bufs=4) as ptpool,
                     tc.tile_pool(name="small", bufs=4) as small,
                 ):
                  for pair in range(NPAIR):
                     qkT_for(pair, ps_s)      # q of this pair
                     qkT_for(2 + pair, ps_s)  # k of this pair
                     for ib in range(n_ib):
                         if pair == 0:
                             # v quarter-blocks on demand: attention for this
                             # ib only needs v key-tiles st <= 4*ib+3
                             v_for(4 * ib, 4 * ib + 4, ps_s)
                         njt = 4 * (ib + 1)  # key tiles needed (j <= i)
                         avs = [
                             ps_av.tile([65, IB], f32, name=f"av{hh}", tag=f"av{hh}")
                             for hh in range(2)
                         ]
                         # diagonal key tiles first: their mask multiply then
                         # overlaps the long non-diagonal score/AV stream
                         jt_order = list(range(4 * ib, njt)) + list(range(4 * ib))
                         for jseq, jt in enumerate(jt_order):
                             pss = ps_s.tile([128, 2 * IB], f32, name="pss")
                             for hh in range(2):
                                 off = hh * 64
                                 nc.tensor.matmul(
                                     pss[:, hh * IB : (hh + 1) * IB],
                                     lhsT=(kT_sb[
                                             off : off + 64,
                                             pair,
                                             jt * 128 : (jt + 1) * 128,
                                         ]
                                     ),
                                     rhs=(qT_sb[
                                             off : off + 64,
                                             pair,
                                             ib * IB : (ib + 1) * IB,
                                         ]
                                     ),
                                     start=True,
                                     stop=True,
                                     tile_position=(off, 0),
                                 )
                             pt = ptpool.tile([128, 2 * IB], f32r, name="pt")
                             nc.scalar.activation(
                                 out=pt, in_=pss, func=Act.Exp, scale=SCALE
                             )
                             k = jt - 4 * ib
                             for hh in range(2):
                                 if k >= 0:  # diagonal tile: causal mask
                                     # split between DVE and the idle gpsimd
                                     eng = nc.vector if hh == 0 else nc.gpsimd
                                     eng.tensor_tensor(
                                         out=pt[:, hh * IB : (hh + 1) * IB],
                                         in0=pt[:, hh * IB : (hh + 1) * IB],
                                         in1=msk_sb[:, k, :],
                                         op=Alu.mult,
                                     )
                                 nc.tensor.matmul(
                                     avs[hh],
                                     lhsT=(v_sb[:, jt, pair * 2 + hh, :]),
                                     rhs=(pt[:, hh * IB : (hh + 1) * IB]),
                                     start=(jseq == 0),
                                     stop=(jseq == njt - 1),
                                 )
                         # normalize per head: aT = av[0:64] * (1 / av[64])
                         for hh in range(2):
                             av = avs[hh]
                             off = hh * 64
                             rec_sb = small.tile([1, IB], f32r, name="rec_sb")
                             with nc.allow_low_precision(
                                 reason="float32r feeds the fp32r replicate matmul"
                             ):
                                 nc.vector.reciprocal(rec_sb, av[64:65, :])
                             # replicate rec across 64 partitions via PE
                             # (reuses a scores-pool slot to stay in 8 banks)
                             rec_ps = ps_s.tile(
                                 [64, IB], f32, name="rec_ps", tag="pss"
                             )
                             nc.tensor.matmul(
                                 rec_ps,
                                 lhsT=(sel2_sb[0:1, 0:64]),
                                 rhs=(rec_sb[:, :]),
                                 start=True,
                                 stop=True,
                             )
                             # HW allows only ONE psum operand per DVE op:
                             # bounce the replicated reciprocal through SBUF.
                             rec_rep = small.tile([64, IB], f32, name="rec_rep")
                             nc.vector.tensor_copy(out=rec_rep, in_=rec_ps)
                             nc.vector.tensor_tensor(
                                 out=aT_sb[
                                     off : off + 64, pair, ib * IB : (ib + 1) * IB
                                 ],
                                 in0=av[0:64, :],
                                 in1=rec_rep,
                                 op=Alu.mult,
                             )
                     if phases != "attn":
                         # stage this pair's AllGather payload immediately
                         nc.sync.dma_start(
                             out=ag_in[pair * 128 : (pair + 1) * 128, :],
                             in_=aT_sb[:, pair, :],
                         )

             if phases == "attn":
                 continue
             # ---------------- phase 3: AllGather + projection ----------------
             nc.gpsimd.collective_compute(
                 "AllGather",
                 Alu.bypass,
                 replica_groups=[[0, 1, 2, 3], [4, 5, 6, 7]],
                 ins=[ag_in[:, :]],
                 outs=[ag_out[:, :]],
             )

             with (
                 tc.tile_pool(name="p3in", bufs=1) as p3in,
                 tc.tile_pool(name="ps_y", bufs=3, space="PSUM") as ps_y,
                 tc.tile_pool(name="yout", bufs=3) as yout,
             ):
                 agf_sb = p3in.tile([128, n_dt, s], bf16)
                 for sh in range(2):
                     for t in range(n_dt):
                         nc.sync.dma_start(
                             out=agf_sb[:, t, sh * s // 2 : (sh + 1) * s // 2],
                             in_=ag_out.rearrange("(t p) ss -> p t ss", p=128)[
                                 :, t, sh * s // 2 : (sh + 1) * s // 2
                             ],
                         )
                 if phases == "ag":
                     continue
                 for st in range(n_st):
                     psy = ps_y.tile([128, OC], f32, name="psy")
                     for t in range(n_dt):
                         nc.tensor.matmul(
                             psy,
                             lhsT=(agf_sb[:, t, st * 128 : (st + 1) * 128]),
                             rhs=(wp_sb[:, t, :]),
                             start=(t == 0),
                             stop=(t == n_dt - 1),
                         )
                     ysb = yout.tile([128, OC], f32, name="ysb")
                     nc.vector.tensor_tensor(
                         out=ysb, in0=psy, in1=bp_sb, op=Alu.add
                     )
                     nc.sync.dma_start(
                         out=y[st * 128 : (st + 1) * 128, :], in_=ysb
                     )

    nc.compile()
    return nc


def _shard_inputs(x, w_attn, b_attn, w_proj, b_proj, s=S):
    """Host-side sharding: build the per-core input maps."""
    import ml_dtypes
    x = np.asarray(x, dtype=np.float32)
    w_attn = np.asarray(w_attn, dtype=np.float32)
    b_attn = np.asarray(b_attn, dtype=np.float32)
    w_proj = np.asarray(w_proj, dtype=np.float32)
    b_proj = np.asarray(b_proj, dtype=np.float32)

    # causal mask tiles: msk[j, k, i] = 1.0 if i >= j + 128*k
    jj = np.arange(128)[:, None, None]
    kk = np.arange(4)[None, :, None]
    ii = np.arange(IB)[None, None, :]
    msk = (ii >= jj + 128 * kk).astype(np.float32)

    in_maps = []
    for core in range(N_CORES):
        b, g = divmod(core, GROUP)
        hs = list(range(g * HLOC, (g + 1) * HLOC))
        xt = np.ascontiguousarray(x[b].T)
        qcols = np.concatenate(
            [w_attn[:, h * HD : (h + 1) * HD] for h in hs], axis=1
        )
        kcols = np.concatenate(
            [w_attn[:, D + h * HD : D + (h + 1) * HD] for h in hs], axis=1
        )
        vcols = np.concatenate(
            [w_attn[:, 2 * D + h * HD : 2 * D + (h + 1) * HD] for h in hs], axis=1
        )
        wqk = np.ascontiguousarray(np.concatenate([qcols, kcols], axis=1))
        wv = np.ascontiguousarray(vcols)
        bq = np.concatenate([b_attn[h * HD : (h + 1) * HD] for h in hs])
        bk = np.concatenate([b_attn[D + h * HD : D + (h + 1) * HD] for h in hs])
        bvv = np.concatenate(
            [b_attn[2 * D + h * HD : 2 * D + (h + 1) * HD] for h in hs]
        )
        bqk = np.concatenate([bq, bk]).reshape(4, 128).T.copy()  # [128, 4]
        bv = np.broadcast_to(bvv, (128, 256)).copy()
        wpc = np.ascontiguousarray(
            w_proj[:, g * OC : (g + 1) * OC]
        ).astype(ml_dtypes.bfloat16)
        bpc = np.broadcast_to(b_proj[g * OC : (g + 1) * OC], (128, OC)).copy()
        in_maps.append(
            dict(
                xt=xt, wqk=wqk, wv=wv, bqk=bqk, bv=bv, wp=wpc, bp=bpc, msk=msk,
                one_r=np.ones((128, 65), np.float32),
                sel2=np.repeat(np.eye(2, dtype=np.float32), 64, axis=1),
            )
        )
    return in_maps


def _unshard(results):
    y = np.empty((B, S, D), np.float32)
    for core in range(N_CORES):
        b, g = divmod(core, GROUP)
        y[b, :, g * OC : (g + 1) * OC] = results[core]["y"]
    return y


_NC_CACHE = {}


def kernel(x, w_attn, b_attn, w_proj, b_proj):
    from concourse.bass_utils import run_bass_kernel_spmd

    if S not in _NC_CACHE:
        _NC_CACHE[S] = _build_bass(S)
    nc = _NC_CACHE[S]
    in_maps = _shard_inputs(x, w_attn, b_attn, w_proj, b_proj)
    res = run_bass_kernel_spmd(nc, in_maps, list(range(N_CORES)))
    return _unshard(res.results)



# revision 14
# speedup vs baseline: 79.0338x; 79.0338x over previous
"""Trainium2 Bass kernel for a GPT-style causal attention block.

  y = proj( softmax_causal( (x@Wq)(x@Wk)^T / sqrt(hd) ) @ (x@Wv) )

Shapes (hardcoded): B=2, S=2048, D=1024, H=16 heads, hd=64.

Sharding over 8 NeuronCores: core = (batch b, head-group g), g selects 4
heads (2 head PAIRS). Single SPMD program; per core:

  for ib in 0..3 (512-query slices; causal, so slice ib attends to
                  keys [0, 512*(ib+1)) ):
      qkT projection for query block ib (chases the x DMA, which lands
          query-block by query-block), v projection for key tiles
          4ib..4ib+3
      deferred normalize + AllGather of the previous slice's pair-1
      attention(ib, pair0), attention(ib, pair1):
          scores in the transposed [key, query] layout, head pairs
          packed into PE row groups; exp on ACT (1/8 scale folded in);
          causal masking multiplies only the [128,128] triangle of each
          diagonal key tile on the GPSIMD engine, and exp/AV are
          restricted to the unmasked query range (full width on ib=0);
          AV with lhsT=[v|1] so psum row 64 accumulates the softmax
          denominator.
      pair0's normalize (DVE reciprocal of the denominator row, PE
          replicate matmul, multiply) is EMBEDDED into pair1's jt loop
          and pair1's into the next iteration's qkT block, so the
          reciprocal latency never stalls the in-order PE queue. Each
          pair's 128-row aT slice is AllGather'd separately (8 small
          collectives total), overlapping the next compute.
      proj(ib-1): output projection of the previous gathered slice,
          transposed (lhsT = Wp tile, 512-moving), accumulating the two
          gathered halves; bias is a per-partition scalar add.

All matmul operands are bf16 (halves PE weight-load time; bf16 moving
operands always run at full PE rate). PSUM accumulation is f32.
Measured relative L2 error vs the fp32 reference: ~4e-3.
"""

import numpy as np

B = 2
S = 2048
D = 1024
H = 16
HD = 64
HLOC = 4          # heads per core
NPAIR = 2         # head pairs per core
N_CORES = 8
GROUP = 4         # cores per batch (replica group size)
IB = 512          # query block width (matmul moving dim)
OC = D // GROUP   # output-projection column shard per core (256)
SCALE = 1.0 / 8.0  # 1/sqrt(hd)


def _build_bass(s=S):
    import concourse.bacc as bacc
    import concourse.mybir as mybir
    import concourse.tile as tile

    f32 = mybir.dt.float32
    f32r = mybir.dt.float32r
    bf16 = mybir.dt.bfloat16
    Alu = mybir.AluOpType
    Act = mybir.ActivationFunctionType

    n_ib = s // IB           # query blocks (4)
    n_st = s // 128          # 128-row sequence tiles (16)
    n_dt = D // 128          # contraction tiles for D (8)

    nc = bacc.Bacc(num_devices=N_CORES)

    xt = nc.declare_dram_parameter("xt", [D, s], bf16, isOutput=False)
    wqk = nc.declare_dram_parameter("wqk", [D, 512], bf16, isOutput=False)
    wv = nc.declare_dram_parameter("wv", [D, 256], bf16, isOutput=False)
    bqk = nc.declare_dram_parameter("bqk", [128, 4], f32, isOutput=False)
    bv = nc.declare_dram_parameter("bv", [128, 256], f32, isOutput=False)
    wp = nc.declare_dram_parameter("wp", [D, OC], bf16, isOutput=False)
    bpt = nc.declare_dram_parameter("bpt", [128, 2], f32, isOutput=False)
    # full-width causal masks for ib=0 (mskf[j,k,:,i] = i >= j + 128k) and
    # the [128,128] triangle (mskt[j,:,c] = c >= j) for diagonal tiles
    mskf = nc.declare_dram_parameter("mskf", [128, 4, 2, IB], bf16, isOutput=False)
    mskt = nc.declare_dram_parameter("mskt", [128, 2, 128], bf16, isOutput=False)
    # selector row for the reciprocal replicate matmul (all-ones [1, 64])
    sel2 = nc.declare_dram_parameter("sel2", [1, 64], f32r, isOutput=False)
    y = nc.declare_dram_parameter("y", [OC, s], f32, isOutput=True)

    with tile.TileContext(nc) as tc:
        with (
            tc.tile_pool(name="const", bufs=1) as const,
            tc.tile_pool(name="persist", bufs=1) as persist,
            tc.tile_pool(name="dram", bufs=1, space="DRAM") as dram,
            tc.tile_pool(name="ps_s", bufs=2, space="PSUM") as ps_s,
            tc.tile_pool(name="ps_av", bufs=2, space="PSUM") as ps_av,
            tc.tile_pool(name="pt", bufs=4) as ptpool,
            tc.tile_pool(name="small", bufs=4) as small,
            tc.tile_pool(name="agf", bufs=4) as agfpool,
            tc.tile_pool(name="yout", bufs=2) as yout,
        ):
            # ---- weights first so the QKV matmuls can start ASAP; x
            # arrives query-block by query-block, interleaved with the
            # tensors each upcoming phase needs next ----
            wqk_sb = const.tile([128, n_dt, 512], bf16)
            nc.sync.dma_start(
                out=wqk_sb, in_=wqk.rearrange("(t p) c -> p t c", p=128)
            )
            bqk_sb = const.tile([128, 4], f32)
            nc.sync.dma_start(out=bqk_sb, in_=bqk[:, :])
            xt_sb = const.tile([128, n_dt, s], bf16)
            xt_r = xt.rearrange("(t p) ss -> p t ss", p=128)

            def load_x(sh):
                for t in range(n_dt):
                    nc.sync.dma_start(
                        out=xt_sb[:, t, sh * IB : (sh + 1) * IB],
                        in_=xt_r[:, t, sh * IB : (sh + 1) * IB],
                    )

            load_x(0)
            wv_sb = const.tile([128, n_dt, 256], bf16)
            nc.sync.dma_start(
                out=wv_sb, in_=wv.rearrange("(t p) c -> p t c", p=128)
            )
            bv_sb = const.tile([128, 256], f32)
            nc.sync.dma_start(out=bv_sb, in_=bv[:, :])
            load_x(1)
            mskf_sb = const.tile([128, 4, 2, IB], bf16)
            nc.sync.dma_start(out=mskf_sb, in_=mskf[:, :, :, :])
            mskt_sb = const.tile([128, 2, 128], bf16)
            nc.sync.dma_start(out=mskt_sb, in_=mskt[:, :, :])
            sel2_sb = const.tile([1, 64], f32r)
            nc.sync.dma_start(out=sel2_sb, in_=sel2[:, :])
            load_x(2)
            wp_sb = const.tile([128, n_dt, OC], bf16)
            nc.sync.dma_start(
                out=wp_sb, in_=wp.rearrange("(t p) c -> p t c", p=128)
            )
            bpt_sb = const.tile([128, 2], f32)
            nc.sync.dma_start(out=bpt_sb, in_=bpt[:, :])
            load_x(3)

            # dummy exp: pulls the ACT exp table load off the critical path
            warm_sb = const.tile([1, 1], f32)
            nc.scalar.activation(
                out=warm_sb, in_=bqk_sb[0:1, 0:1], func=Act.Exp, scale=0.0
            )

            # persistent intermediates (all bf16)
            qT_sb = persist.tile([128, NPAIR, s], bf16)   # [pair_ch, pair, s]
            kT_sb = persist.tile([128, NPAIR, s], bf16)
            v_sb = persist.tile([128, n_st, HLOC, 65], bf16)
            aT_sb = persist.tile([128, NPAIR, s], bf16)

            # ones column for the softmax-denominator trick
            nc.gpsimd.memset(v_sb[:, :, :, 64:65], 1.0)

            ag_in = [
                [dram.tile([128, IB], bf16, name=f"ag_in{i}_{p}") for p in range(2)]
                for i in range(n_ib)
            ]
            ag_out = [
                [dram.tile([512, IB], bf16, name=f"ag_out{i}_{p}") for p in range(2)]
                for i in range(n_ib)
            ]

            def qkT_v(ib):
                # c-tile t: 0,1 = q pair0/1; 2,3 = k pair0/1
                for t in range(4):
                    ps = ps_s.tile([128, IB], f32, name="ps", tag="pss")
                    for dt in range(n_dt):
                        nc.tensor.matmul(
                            ps,
                            lhsT=(wqk_sb[:, dt, t * 128 : (t + 1) * 128]),
                            rhs=(xt_sb[:, dt, ib * IB : (ib + 1) * IB]),
                            start=(dt == 0),
                            stop=(dt == n_dt - 1),
                        )
                    dst = qT_sb if t < 2 else kT_sb
                    nc.vector.tensor_scalar_add(
                        out=dst[:, t % 2, ib * IB : (ib + 1) * IB],
                        in0=ps,
                        scalar1=bqk_sb[:, t : t + 1],
                    )
                for st in range(4 * ib, 4 * ib + 4):
                    psv = ps_s.tile([128, 256], f32, name="psv", tag="pss")
                    for dt in range(n_dt):
                        nc.tensor.matmul(
                            psv,
                            lhsT=(xt_sb[:, dt, st * 128 : (st + 1) * 128]),
                            rhs=(wv_sb[:, dt, :]),
                            start=(dt == 0),
                            stop=(dt == n_dt - 1),
                        )
                    nc.vector.tensor_tensor(
                        out=v_sb[:, st, :, 0:64],
                        in0=psv.rearrange("p (h e) -> p h e", h=HLOC),
                        in1=bv_sb.rearrange("p (h e) -> p h e", h=HLOC),
                        op=Alu.add,
                    )

            def norm_recips(st_):
                """DVE reciprocals of the two denominator rows (start early)."""
                ib, pair, avs = st_["ib"], st_["pair"], st_["avs"]
                for hh in range(2):
                    rec1 = small.tile([1, IB], f32r, name="rec1")
                    with nc.allow_low_precision(
                        reason="float32r feeds the fp32r replicate matmul"
                    ):
                        nc.vector.reciprocal(out=rec1, in_=avs[hh][64:65, :])
                    st_["rec1"].append(rec1)

            def norm_repmm(st_, hh):
                """PE replicate matmul for one head (emit as PE filler)."""
                rec_ps = ps_s.tile([64, IB], f32, name="rec_ps", tag="pss")
                nc.tensor.matmul(
                    rec_ps,
                    lhsT=(sel2_sb[0:1, 0:64]),
                    rhs=(st_["rec1"][hh][:, :]),
                    start=True,
                    stop=True,
                )
                st_["rec_ps"].append(rec_ps)

            def norm_finish(st_):
                """Copies + multiplies + stage + collective for one pair."""
                ib, pair, avs = st_["ib"], st_["pair"], st_["avs"]
                rec_rep = small.tile([128, IB], f32, name="rec_rep")
                for hh in range(2):
                    nc.vector.tensor_copy(
                        out=rec_rep[hh * 64 : (hh + 1) * 64, :],
                        in_=st_["rec_ps"][hh],
                    )
                for hh in range(2):
                    poff = hh * 64
                    nc.vector.tensor_tensor(
                        out=aT_sb[poff : poff + 64, pair, ib * IB : (ib + 1) * IB],
                        in0=avs[hh][0:64, :],
                        in1=rec_rep[poff : poff + 64, :],
                        op=Alu.mult,
                    )
                nc.sync.dma_start(
                    out=ag_in[ib][pair][:, :],
                    in_=aT_sb[:, pair, ib * IB : (ib + 1) * IB],
                )
                nc.gpsimd.collective_compute(
                    "AllGather",
                    Alu.bypass,
                    replica_groups=[[0, 1, 2, 3], [4, 5, 6, 7]],
                    ins=[ag_in[ib][pair][:, :]],
                    outs=[ag_out[ib][pair][:, :]],
                )

            def attention(ib, pair):
                """Scores/exp/mask/AV for one (query block, head pair).
                Ends by starting the denominator reciprocals on the DVE;
                the PE side of the normalize is emitted later (after the
                next block of PE work) so it never stalls the PE queue."""
                njt = 4 * (ib + 1)
                avs = [
                    ps_av.tile([65, IB], f32, name=f"av{hh}", tag=f"av{hh}")
                    for hh in range(2)
                ]
                jt_order = list(range(4 * ib, njt)) + list(range(4 * ib))
                pend = None  # software-pipelined AV emission
                for jseq, jt in enumerate(jt_order):
                    k = jt - 4 * ib
                    off = 128 * k if (ib > 0 and k >= 1) else 0
                    pss = ps_s.tile([128, 2, IB], f32, name="pss", tag="pss")
                    for hh in range(2):
                        poff = hh * 64
                        nc.tensor.matmul(
                            pss[:, hh, off:],
                            lhsT=(kT_sb[
                                    poff : poff + 64, pair,
                                    jt * 128 : (jt + 1) * 128,
                                ]),
                            rhs=(qT_sb[
                                    poff : poff + 64, pair,
                                    ib * IB + off : (ib + 1) * IB,
                                ]),
                            start=True,
                            stop=True,
                            tile_position=(poff, 0),
                        )
                    pt = ptpool.tile([128, 2, IB], bf16, name="pt")
                    nc.scalar.activation(
                        out=pt[:, :, off:], in_=pss[:, :, off:],
                        func=Act.Exp, scale=SCALE,
                    )
                    if k >= 0:  # diagonal tile: causal mask (on gpsimd)
                        if ib > 0 or k == 0:
                            nc.gpsimd.tensor_tensor(
                                out=pt[:, :, off : off + 128],
                                in0=pt[:, :, off : off + 128],
                                in1=mskt_sb,
                                op=Alu.mult,
                            )
                        else:  # ib == 0, k >= 1: full-width mask
                            nc.gpsimd.tensor_tensor(
                                out=pt, in0=pt, in1=mskf_sb[:, k, :, :],
                                op=Alu.mult,
                            )
                    if pend is not None:
                        for mm in pend:
                            nc.tensor.matmul(**mm)
                    pend = [
                        dict(
                            out=avs[hh][:, off:],
                            lhsT=(v_sb[:, jt, pair * 2 + hh, :]),
                            rhs=(pt[:, hh, off:]),
                            start=(jseq == 0),
                            stop=(jseq == njt - 1),
                        )
                        for hh in range(2)
                    ]
                for mm in pend:
                    nc.tensor.matmul(**mm)
                st_ = dict(ib=ib, pair=pair, avs=avs, rec1=[], rec_ps=[])
                norm_recips(st_)
                return st_

            def norm_pe(st_):
                for hh in range(2):
                    norm_repmm(st_, hh)
                norm_finish(st_)

            def proj_slice(ib):
                """Output projection for gathered slice ib:
                yT[oc, q] = sum_c Wp[c, oc] agT[c, q], accumulating the
                two gathered pair-halves."""
                agfs = []
                for part in range(2):
                    agf_sb = agfpool.tile(
                        [128, 4, IB], bf16, name=f"agf{part}", tag="agf"
                    )
                    nc.sync.dma_start(
                        out=agf_sb,
                        in_=ag_out[ib][part].rearrange("(t p) q -> p t q", p=128),
                    )
                    agfs.append(agf_sb)
                psy = [
                    ps_s.tile([128, IB], f32, name=f"psy{h}", tag="pss")
                    for h in range(2)
                ]
                for part in range(2):
                    for h in range(2):
                        for g in range(4):
                            nc.tensor.matmul(
                                psy[h],
                                lhsT=(wp_sb[:, 2 * g + part, h * 128 : (h + 1) * 128]),
                                rhs=(agfs[part][:, g, :]),
                                start=(part == 0 and g == 0),
                                stop=(part == 1 and g == 3),
                            )
                for h in range(2):
                    ysb = yout.tile([128, IB], f32, name="ysb")
                    nc.vector.tensor_scalar_add(
                        out=ysb, in0=psy[h], scalar1=bpt_sb[:, h : h + 1]
                    )
                    nc.sync.dma_start(
                        out=y[h * 128 : (h + 1) * 128, ib * IB : (ib + 1) * IB],
                        in_=ysb,
                    )

            pending = None  # pair-1 normalize state, deferred one iteration
            for ib in range(n_ib):
                qkT_v(ib)
                if pending is not None:
                    norm_pe(pending)
                st0 = attention(ib, 0)
                pending_new = attention(ib, 1)
                norm_pe(st0)
                if ib > 0:
                    proj_slice(ib - 1)
                pending = pending_new
            norm_pe(pending)
            proj_slice(n_ib - 1)

    nc.compile()
    return nc


def _shard_inputs(x, w_attn, b_attn, w_proj, b_proj, s=S):
    """Host-side sharding: build the per-core input maps."""
    import ml_dtypes
    bf16 = ml_dtypes.bfloat16
    x = np.asarray(x, dtype=np.float32)
    w_attn = np.asarray(w_attn, dtype=np.float32)
    b_attn = np.asarray(b_attn, dtype=np.float32)
    w_proj = np.asarray(w_proj, dtype=np.float32)
    b_proj = np.asarray(b_proj, dtype=np.float32)

    # full-width causal mask tiles for ib=0: mskf[j, k, :, i] = 1.0 if i >= j+128k
    jj = np.arange(128)[:, None, None, None]
    kk = np.arange(4)[None, :, None, None]
    ii = np.arange(IB)[None, None, None, :]
    mskf = np.broadcast_to(ii >= jj + 128 * kk, (128, 4, 2, IB)).astype(bf16)
    # triangle mask (duplicated for the 2-head layout)
    mskt = np.broadcast_to(
        (np.arange(128)[None, None, :] >= np.arange(128)[:, None, None]),
        (128, 2, 128),
    ).astype(bf16)

    in_maps = []
    for core in range(N_CORES):
        b, g = divmod(core, GROUP)
        hs = list(range(g * HLOC, (g + 1) * HLOC))
        xt = np.ascontiguousarray(x[b].T).astype(bf16)
        qcols = np.concatenate(
            [w_attn[:, h * HD : (h + 1) * HD] for h in hs], axis=1
        )
        kcols = np.concatenate(
            [w_attn[:, D + h * HD : D + (h + 1) * HD] for h in hs], axis=1
        )
        vcols = np.concatenate(
            [w_attn[:, 2 * D + h * HD : 2 * D + (h + 1) * HD] for h in hs], axis=1
        )
        wqk = np.ascontiguousarray(
            np.concatenate([qcols, kcols], axis=1)
        ).astype(bf16)
        wv = np.ascontiguousarray(vcols).astype(bf16)
        bq = np.concatenate([b_attn[h * HD : (h + 1) * HD] for h in hs])
        bk = np.concatenate([b_attn[D + h * HD : D + (h + 1) * HD] for h in hs])
        bvv = np.concatenate(
            [b_attn[2 * D + h * HD : 2 * D + (h + 1) * HD] for h in hs]
        )
        bqk = np.concatenate([bq, bk]).reshape(4, 128).T.copy()  # [128, 4]
        bv = np.broadcast_to(bvv, (128, 256)).copy()
        wpc = np.ascontiguousarray(w_proj[:, g * OC : (g + 1) * OC]).astype(bf16)
        bpt = np.ascontiguousarray(
            b_proj[g * OC : (g + 1) * OC].reshape(2, 128).T
        )  # bpt[p, h] = b_proj[g*256 + 128h + p]
        in_maps.append(
            dict(
                xt=xt, wqk=wqk, wv=wv, bqk=bqk, bv=bv, wp=wpc, bpt=bpt,
                mskf=mskf, mskt=mskt, sel2=np.ones((1, 64), np.float32),
            )
        )
    return in_maps


def _unshard(results):
    y = np.empty((B, S, D), np.float32)
    for core in range(N_CORES):
        b, g = divmod(core, GROUP)
        y[b, :, g * OC : (g + 1) * OC] = results[core]["y"].T
    return y


_NC_CACHE = {}


def kernel(x, w_attn, b_attn, w_proj, b_proj):
    from concourse.bass_utils import run_bass_kernel_spmd

    if S not in _NC_CACHE:
        _NC_CACHE[S] = _build_bass(S)
    nc = _NC_CACHE[S]
    in_maps = _shard_inputs(x, w_attn, b_attn, w_proj, b_proj)
    res = run_bass_kernel_spmd(nc, in_maps, list(range(N_CORES)))
    return _unshard(res.results)
